# revision 16
# baseline (speedup 1.0000x reference)
"""AutoQuantConv2d Trainium2 kernel.

Computes conv2d(fake_quant_nvfp4(x), fake_quant_nvfp4(w)) for
x [32,256,64,64] f32, w [256,256,3,3] f32, stride 1, pad 1, NCHW/OIHW.

Sharding: data-parallel over batch — each of the 8 NeuronCores gets 4
images and the full weight; outputs are concatenated on host.

On-core pipeline (v2 — PE-bound schedule):
  1. NVFP4 fake-quant, done exactly in fp32 bit arithmetic (no division):
       amax   = blocked absmax (16 contiguous elements)        [reduce]
       scale  = 2*floor_pow2(max(amax/6, eps))                 [3 small ops]
       q      = (v + t) - t,  t = max(v & 0x7f800000, scale) * 3*2^21
     The last line runs as ONE custom fused DVE op; q is written directly
     as fp8e4 (weights are prescaled by 2^8 on the Scalar engine so every
     q*scale stays normal in e4m3; the 1/256 descale rides the PSUM drain).
  2. Weight lhsT tiles are produced by PE transposes (fp8 matmul with an
     identity moving operand) into PSUM, then copied to SBUF by ScalarE.
     No DMA transposes; the PE warms its p-state before the conv starts.
  3. conv2d as implicit GEMM, bank-sequential: per (image, 128-out-chan
     chunk, 8-row block) one PSUM bank accumulates 9 DoubleRow matmuls
     [K=256 folded, M=128, N=512]; banks drain (ScalarE, bf16, x1/256)
     while the PE fills the next bank, so the PE never waits on drains.
  4. Input DMAs ride the GpSimd queue, output DMAs the Sync queue — the
     two streams can't head-of-line block each other. Output is stored
     bf16 (halves store traffic); host upcasts to f32.
"""

import numpy as np

import concourse.bass as bass
import concourse.mybir as mybir
from concourse.tile import TileContext
from concourse.bass_utils import run_bass_kernel_spmd
from contextlib import ExitStack

AO = mybir.AluOpType
F32 = mybir.dt.float32
I32 = mybir.dt.int32
BF16 = mybir.dt.bfloat16
FP8 = mybir.dt.float8e4

N_CORES = 8
N_PER = 4          # images per core
C = 256            # input channels
O = 256            # output channels
H = W = 64
HP = WP = 66       # padded spatial
F = H * W          # 4096 pixels per channel
NB = F // 16       # 256 quant blocks per channel row
KF = C * 9         # 2304 flattened weight row per output channel
WNB = KF // 16     # 144 quant blocks per weight row

MASK_EXP = 0x7F800000
K_MAGIC = 6291456.0  # 3 * 2^21: t = floor_pow2(max(|v|,scale)) * K is the round magic

H0 = 34            # rows in the first half-plane of image 0 (split quant)


# ---------------------------------------------------------------------------
# custom fused DVE op: q = (v + t) - t, t = max(v & expmask, scale) * K
# ---------------------------------------------------------------------------
def _get_fused_quant_op():
    from concourse.dve_ops import OPS, DveOp
    import concourse.dve_ops as dm
    from concourse.dve_spec import Spec, Src0, Src1, Bin, lower, maxx, _has_src1, C0, C1
    from concourse.dve_uop import DveOpSpec, AluOp

    name = "ANT_NVFP4_FUSED"
    for op in OPS:
        if op.name == name:
            return op
    t = Bin(AluOp.MULTIPLY, maxx(Bin(AluOp.BITWISE_AND, Src0, C0), Src1), C1)
    spec = Spec(
        body=Bin(AluOp.SUBTRACT, Bin(AluOp.ADD, Src0, t), t),
        reference=lambda in0, in1, s0, s1, imm2: in0,
    )
    shas = {}
    for ver in ("v3", "v4"):
        uops = lower(spec, ver=ver)
        shas[ver] = DveOpSpec(name=name, uops=uops, rd1_en=_has_src1(spec)).sha(ver)
    op = DveOp(name, spec, False, uops_sha=shas)
    OPS.append(op)
    dm._SUB_OPCODE_FOR_NAME[name] = dm._CUSTOM_DVE_ROW_BASE + len(OPS) - 1
    return op


def _split_waits(nc, maxw=1):
    """walrus here rejects >1 sync-wait per instruction; hoist extras onto
    preceding same-engine NOPs."""
    bbs = []
    for fn in nc.m.functions:
        for bb in fn.blocks:
            bbs.append((bb, list(bb.instructions)))
    new_lists = []
    for bb, insts in bbs:
        out = []
        for inst in insts:
            si = inst.sync_info
            waits = list(si.on_wait) if si and si.on_wait else []
            if len(waits) > maxw:
                chunks = [waits[i : i + maxw] for i in range(0, len(waits), maxw)]
                eng = nc.engines[inst.engine]
                for chunk in chunks[:-1]:
                    bi = eng.nop(nofuse=True)
                    ni = bi.ins if hasattr(bi, "ins") else bi
                    ni.sync_info = mybir.SyncInfo(on_wait=chunk, on_update=[])
                    out.append(ni)
                inst.sync_info = mybir.SyncInfo(
                    on_wait=chunks[-1], on_update=list(si.on_update or [])
                )
            out.append(inst)
        new_lists.append((bb, out))
    for bb, out in new_lists:
        bb.instructions = out


def _emit_quant(nc, qop, maskt, xd, nblocks, amax, out_ap):
    """NVFP4 fake-quant of SBUF AP xd [128, nblocks*16] f32 into out_ap."""
    xd = xd[:, :]
    nc.vector.tensor_reduce(
        amax[:, :],
        xd.rearrange("p (b s) -> p b s", s=16),
        axis=mybir.AxisListType.X,
        op=AO.max,
        apply_absolute_value=True,
    )
    # scale bits = ((max(amax/6, eps)) & expmask) + 1<<23   (pow2, exact)
    nc.vector.tensor_scalar(amax[:, :], amax[:, :], 1.0 / 6.0, 6e-31, AO.mult, AO.max)
    am_i = amax[:, :].bitcast(I32)
    nc.vector.tensor_scalar(am_i, am_i, MASK_EXP, None, AO.bitwise_and)
    nc.vector.tensor_scalar(am_i, am_i, 0x00800000, None, AO.add)
    nc.vector._custom_dve(
        qop,
        out=out_ap,
        in0=xd.rearrange("p (b s) -> p b s", s=16),
        in1=amax[:, :].broadcast_to([128, nblocks, 16]),
        s0=maskt[:, :],
        s1=K_MAGIC,
    )


# bisect flags (module-level so a driver can toggle before _build)
USE_PE_TRANSPOSE = True   # else: DMA-transpose + gpsimd cast (baseline style)
IN_DMA_GPSIMD = True      # else: input DMAs on the Sync queue
OUT_BF16 = True           # else: f32 output


def _build():
    qop = _get_fused_quant_op()
    nc = bass.Bass(trn_type="TRN2")
    x = nc.dram_tensor("x", [N_PER, C, H, W], F32, kind="ExternalInput")
    w = nc.dram_tensor("w", [O, C, 3, 3], F32, kind="ExternalInput")
    ident = nc.dram_tensor("ident", [128, 128], BF16, kind="ExternalInput")
    out = nc.dram_tensor(
        "out", [N_PER, O, H, W], BF16 if OUT_BF16 else F32, kind="ExternalOutput"
    )

    FPLANE = 4368  # 66*66 padded to a multiple of 16 (DoubleRow step constraint)
    ring = 3

    with TileContext(nc) as tc:
        with ExitStack() as ctx:
            wpool = ctx.enter_context(tc.tile_pool(name="wpool", bufs=1))
            lpool = ctx.enter_context(tc.tile_pool(name="lpool", bufs=1))
            xqpool = ctx.enter_context(tc.tile_pool(name="xqpool", bufs=1))
            xdpool = ctx.enter_context(tc.tile_pool(name="xdpool", bufs=4))
            smpool = ctx.enter_context(tc.tile_pool(name="smpool", bufs=2))
            obpool = ctx.enter_context(tc.tile_pool(name="obpool", bufs=8))
            pspool = ctx.enter_context(tc.tile_pool(name="ps", bufs=5, space="PSUM"))
            tppool = ctx.enter_context(tc.tile_pool(name="tp", bufs=1, space="PSUM"))

            ieng = nc.gpsimd if IN_DMA_GPSIMD else nc.sync

            maskt = wpool.tile([128, 1], F32, name="maskt", tag="maskt")
            nc.vector.memset(maskt[:, :].bitcast(I32), MASK_EXP)

            idt2 = wpool.tile([128, 128], BF16, name="idt2", tag="idt2")
            scr = wpool.tile([128, 128], BF16, name="scr", tag="scr")

            # ---- input DMAs all ride the GpSimd queue (nothing else runs
            # there), so output stores on Sync can't head-of-line block them.
            wf = [None, None]
            for oc in range(2):
                wf[oc] = wpool.tile([128, KF], F32, name=f"wf{oc}", tag=f"wf{oc}")
            xds = {}

            def emit_x_dma(n, c, halves=False):
                xd = xdpool.tile([128, F], F32, name=f"xd_{n}_{c}", tag="xd")
                src = x[n, c * 128 : (c + 1) * 128, :, :].rearrange("c h w -> c (h w)")
                if halves:
                    ieng.dma_start(out=xd[:, 0 : H0 * W], in_=src[:, 0 : H0 * W])
                    ieng.dma_start(out=xd[:, H0 * W : F], in_=src[:, H0 * W : F])
                else:
                    ieng.dma_start(out=xd[:, :], in_=src)
                xds[(n, c)] = xd

            # xq ring tiles; zero image-0's border first (tiny, no deps),
            # the other rings' borders after the early DMA issues
            xq_tiles = []

            def emit_border_memset(t):
                tv = t[:, :, 0 : HP * WP].rearrange("p c (h w) -> p c h w", h=HP)
                nc.gpsimd.memset(tv[:, :, 0, :], 0.0)
                nc.gpsimd.memset(tv[:, :, HP - 1, :], 0.0)
                nc.gpsimd.memset(tv[:, :, 1 : HP - 1, 0], 0.0)
                nc.gpsimd.memset(tv[:, :, 1 : HP - 1, WP - 1], 0.0)

            for r in range(ring):
                t = xqpool.tile([128, 2, FPLANE], FP8, name=f"xq{r}", tag=f"xq{r}")
                xq_tiles.append(t)

            # strict priority order on the single input queue: w0 in two
            # ic-halves, the x0 top row-halves, ident, then the prefetches
            ieng.dma_start(out=idt2[:, :], in_=ident[:, :])
            wsrc0 = w[0:128, :, :, :].rearrange("o i kh kw -> o (i kh kw)")
            KF4 = KF // 4
            for k in range(4):
                ieng.dma_start(
                    out=wf[0][:, k * KF4 : (k + 1) * KF4],
                    in_=wsrc0[:, k * KF4 : (k + 1) * KF4],
                )
            xd00 = xdpool.tile([128, F], F32, name="xd_0_0", tag="xd")
            xd01 = xdpool.tile([128, F], F32, name="xd_0_1", tag="xd")
            xds[(0, 0)], xds[(0, 1)] = xd00, xd01
            srcs = [
                x[0, c * 128 : (c + 1) * 128, :, :].rearrange("c h w -> c (h w)")
                for c in range(2)
            ]
            for c, xd in ((0, xd00), (1, xd01)):
                ieng.dma_start(out=xd[:, 0 : H0 * W], in_=srcs[c][:, 0 : H0 * W])
            emit_border_memset(xq_tiles[0])
            for c, xd in ((0, xd00), (1, xd01)):
                ieng.dma_start(out=xd[:, H0 * W : F], in_=srcs[c][:, H0 * W : F])
            ieng.dma_start(
                out=wf[1][:, :],
                in_=w[128:256, :, :, :].rearrange("o i kh kw -> o (i kh kw)"),
            )
            emit_x_dma(1, 0)
            emit_x_dma(1, 1)
            for r in range(1, ring):
                emit_border_memset(xq_tiles[r])
            for n in (2, 3):
                for c in range(2):
                    emit_x_dma(n, c)

            # ---- weight quant: prescale x256 on ScalarE, quant on DVE (fp8
            # out), then PE-transpose into DoubleRow lhsT tiles.
            wq = [None, None]

            def emit_w_quant(oc, quarters=False):
                if not quarters:
                    wqd = wpool.tile([128, KF], BF16, name=f"wqd{oc}", tag=f"wqd{oc}")
                    wam = smpool.tile([128, WNB], F32, name=f"wam{oc}", tag="amax")
                    _emit_quant(nc, qop, maskt, wf[oc], WNB, wam, wqd[:, :])
                    wq[oc] = [
                        wqd[:, :].rearrange("p (i k) -> p k i", k=9),
                        wqd[:, :].rearrange("p (i k) -> p k i", k=9),
                    ]
                    wq[oc][1] = None  # single-tile: slice by ic at use site
                    wq[oc] = (wqd, None)
                    return
                # two ic-half tiles so the transposes' strided reads depend
                # only on their half (region tracking is per-tile)
                h = KF // 2
                wqa = wpool.tile([128, h], BF16, name=f"wqa{oc}", tag=f"wqa{oc}")
                wqb = wpool.tile([128, h], BF16, name=f"wqb{oc}", tag=f"wqb{oc}")
                for k, dst in ((0, wqa), (1, wqa), (2, wqb), (3, wqb)):
                    wam = smpool.tile(
                        [128, WNB // 4], F32, name=f"wam{oc}_{k}", tag="amax"
                    )
                    o = (k % 2) * (h // 2)
                    _emit_quant(
                        nc, qop, maskt, wf[oc][:, k * (h // 2) : (k + 1) * (h // 2)],
                        WNB // 4, wam, dst[:, o : o + h // 2],
                    )
                wq[oc] = (wqa, wqb)

            lhsT = {}
            tps = [
                tppool.tile([128, 1024], BF16, name="tpA", tag="tpA"),
                tppool.tile([128, 1024], BF16, name="tpB", tag="tpB"),
                tppool.tile([128, 1024], BF16, name="tpC", tag="tpC"),
            ]

            def emit_w_transposes(oc):
                wqa, wqb = wq[oc]
                if wqb is None:
                    s9 = wqa[:, :].rearrange("p (i k) -> p k i", k=9)
                    srcs9 = [s9[:, :, 0:128], s9[:, :, 128:256]]
                else:
                    srcs9 = [
                        t[:, :].rearrange("p (i k) -> p k i", k=9)
                        for t in (wqa, wqb)
                    ]
                ltall = lpool.tile(
                    [128, 9, 2, 128], FP8, name=f"lt{oc}", tag=f"lt{oc}"
                )
                for t9 in range(9):
                    lhsT[(t9 // 3, t9 % 3, oc)] = ltall[:, t9, :, :]
                for ic in range(2):
                    for t9 in range(9):
                        tp = tps[t9 // 4]
                        r = (t9 % 4) * 256
                        nc.tensor.matmul(
                            tp[:, r + ic * 128 : r + ic * 128 + 128],
                            srcs9[ic][:, t9, :],
                            idt2[:, :],
                            is_transpose=True,
                        )
                # three batched PSUM->SBUF copies (one per bank); the x256
                # weight prescale rides the activation scale
                for b, (t0, t1) in enumerate(((0, 4), (4, 8), (8, 9))):
                    nc.scalar.activation(
                        ltall[:, t0:t1, :, :], tps[b][:, 0 : (t1 - t0) * 256],
                        mybir.ActivationFunctionType.Copy, scale=256.0,
                    )

            nc.vector.memset(scr[:, :], 0.0)
            for i in range(60):
                nc.tensor.matmul(
                    tps[2][:, 768:896], scr[:, :], idt2[:, :],
                    is_transpose=True, skip_group_check=(i > 0),
                )
            emit_w_quant(0, quarters=True)
            emit_w_transposes(0)

            # n=0 quant in row-slices (10/24/30 rows) so the first conv
            # group only waits for rows 0..10; oc=1 weight quant rides after
            xq0 = xq_tiles[0]
            xq0v = xq0[:, :, 0 : HP * WP].rearrange("p c (h w) -> p c h w", h=HP)
            for r0, r1 in ((0, 10), (10, H0), (H0, 50), (50, H)):
                for c in range(2):
                    nb = (r1 - r0) * 4
                    am = smpool.tile(
                        [128, nb], F32, name=f"amax_0_{c}_{r0}", tag="amax"
                    )
                    _emit_quant(
                        nc, qop, maskt, xds[(0, c)][:, r0 * W : r1 * W], nb, am,
                        xq0v[:, c, r0 + 1 : r1 + 1, 1 : W + 1],
                    )
            emit_w_quant(1)

            # ---- main loop: bank-sequential implicit GEMM ----
            def emit_group(n, oc, hb, xqv):
                ps = pspool.tile([128, 512], F32, name=f"ps_{n}_{oc}_{hb}", tag="ps")
                k = 0
                for kh in range(3):
                    for kw in range(3):
                        rhs = xqv[:, :, hb * 8 + kh : hb * 8 + kh + 8, kw : kw + 64]
                        nc.tensor.matmul(
                            ps[:, :],
                            lhsT[(kh, kw, oc)][:, :, :],
                            rhs,
                            start=(k == 0),
                            stop=(k == 8),
                            perf_mode=mybir.MatmulPerfMode.DoubleRow,
                        )
                        k += 1
                ob = obpool.tile([128, 512], BF16, name=f"ob_{n}_{oc}_{hb}", tag="ob")
                # descale the 2^8 weight prescale on the way out
                nc.scalar.activation(
                    ob[:, :], ps[:, :],
                    mybir.ActivationFunctionType.Copy, scale=1.0 / 256.0,
                )
                nc.sync.dma_start(
                    out=out[n, oc * 128 : (oc + 1) * 128, hb * 8 : hb * 8 + 8, :],
                    in_=ob[:, :].rearrange("p (h w) -> p h w", h=8),
                )

            for n in range(N_PER):
                xq = xq_tiles[n % ring]
                xqv = xq[:, :, 0 : HP * WP].rearrange("p c (h w) -> p c h w", h=HP)
                if n > 0:
                    for r0, r1 in ((0, H0), (H0, H)):
                        for c in range(2):
                            nb = (r1 - r0) * 4
                            amax = smpool.tile(
                                [128, nb], F32, name=f"amax_{n}_{c}_{r0}", tag="amax"
                            )
                            _emit_quant(
                                nc, qop, maskt,
                                xds[(n, c)][:, r0 * W : r1 * W], nb, amax,
                                xqv[:, c, r0 + 1 : r1 + 1, 1 : W + 1],
                            )
                for oc in range(2):
                    for hb in range(8):
                        emit_group(n, oc, hb, xqv)
                    if n == 0 and oc == 0:
                        # oc=1 lhsT prep rides after image 0's first chunk
                        emit_w_transposes(1)

    mybir.codegen_inst_isa_subclasses(nc)
    _split_waits(nc, maxw=1)
    return nc


_NC_CACHE = None


def _get_nc():
    global _NC_CACHE
    if _NC_CACHE is None:
        _NC_CACHE = _build()
    return _NC_CACHE


def _ident_np():
    import ml_dtypes

    return np.eye(128, dtype=np.float32).astype(ml_dtypes.bfloat16)


def _in_maps(x, w):
    ident = _ident_np()
    return [
        {"x": x[i * N_PER : (i + 1) * N_PER], "w": w, "ident": ident}
        for i in range(N_CORES)
    ]


def kernel(x: np.ndarray, w: np.ndarray) -> np.ndarray:
    x = np.ascontiguousarray(x, dtype=np.float32)
    w = np.ascontiguousarray(w, dtype=np.float32)
    nc = _get_nc()
    res = run_bass_kernel_spmd(nc, _in_maps(x, w), core_ids=list(range(N_CORES)))
    outs = [
        np.asarray(res.results[i]["out"]).astype(np.float32)
        for i in range(N_CORES)
    ]
    return np.concatenate(outs, axis=0)


# revision 17
# speedup vs baseline: 1.1883x; 1.1883x over previous
"""AutoQuantConv2d Trainium2 kernel.

Computes conv2d(fake_quant_nvfp4(x), fake_quant_nvfp4(w)) for
x [32,256,64,64] f32, w [256,256,3,3] f32, stride 1, pad 1, NCHW/OIHW.

Sharding: data-parallel over batch — each of the 8 NeuronCores gets 4
images and the full weight; outputs are concatenated on host.

On-core pipeline (v2 — PE-bound schedule):
  1. NVFP4 fake-quant, done exactly in fp32 bit arithmetic (no division):
       amax   = blocked absmax (16 contiguous elements)        [reduce]
       scale  = 2*floor_pow2(max(amax/6, eps))                 [3 small ops]
       q      = (v + t) - t,  t = max(v & 0x7f800000, scale) * 3*2^21
     The last line runs as ONE custom fused DVE op; q is written directly
     as fp8e4 (weights are prescaled by 2^8 on the Scalar engine so every
     q*scale stays normal in e4m3; the 1/256 descale rides the PSUM drain).
  2. Weight lhsT tiles are produced by PE transposes (fp8 matmul with an
     identity moving operand) into PSUM, then copied to SBUF by ScalarE.
     No DMA transposes; the PE warms its p-state before the conv starts.
  3. conv2d as implicit GEMM, bank-sequential: per (image, 128-out-chan
     chunk, 8-row block) one PSUM bank accumulates 9 DoubleRow matmuls
     [K=256 folded, M=128, N=512]; banks drain (ScalarE, bf16, x1/256)
     while the PE fills the next bank, so the PE never waits on drains.
  4. Input DMAs ride the GpSimd queue, output DMAs the Sync queue — the
     two streams can't head-of-line block each other. Output is stored
     bf16 (halves store traffic); host upcasts to f32.
"""

import numpy as np

import concourse.bass as bass
import concourse.mybir as mybir
from concourse.tile import TileContext
from concourse.bass_utils import run_bass_kernel_spmd
from contextlib import ExitStack

AO = mybir.AluOpType
F32 = mybir.dt.float32
I32 = mybir.dt.int32
BF16 = mybir.dt.bfloat16
FP8 = mybir.dt.float8e4

N_CORES = 8
N_PER = 4          # images per core
C = 256            # input channels
O = 256            # output channels
H = W = 64
HP = WP = 66       # padded spatial
F = H * W          # 4096 pixels per channel
NB = F // 16       # 256 quant blocks per channel row
KF = C * 9         # 2304 flattened weight row per output channel
WNB = KF // 16     # 144 quant blocks per weight row

MASK_EXP = 0x7F800000
K_MAGIC = 6291456.0  # 3 * 2^21: t = floor_pow2(max(|v|,scale)) * K is the round magic

H0 = 34            # rows in the first half-plane of image 0 (split quant)


# ---------------------------------------------------------------------------
# custom fused DVE op: q = (v + t) - t, t = max(v & expmask, scale) * K
# ---------------------------------------------------------------------------
def _get_fused_quant_op():
    from concourse.dve_ops import OPS, DveOp
    import concourse.dve_ops as dm
    from concourse.dve_spec import Spec, Src0, Src1, Bin, lower, maxx, _has_src1, C0, C1
    from concourse.dve_uop import DveOpSpec, AluOp

    name = "ANT_NVFP4_FUSED"
    for op in OPS:
        if op.name == name:
            return op
    t = Bin(AluOp.MULTIPLY, maxx(Bin(AluOp.BITWISE_AND, Src0, C0), Src1), C1)
    spec = Spec(
        body=Bin(AluOp.SUBTRACT, Bin(AluOp.ADD, Src0, t), t),
        reference=lambda in0, in1, s0, s1, imm2: in0,
    )
    shas = {}
    for ver in ("v3", "v4"):
        uops = lower(spec, ver=ver)
        shas[ver] = DveOpSpec(name=name, uops=uops, rd1_en=_has_src1(spec)).sha(ver)
    op = DveOp(name, spec, False, uops_sha=shas)
    OPS.append(op)
    dm._SUB_OPCODE_FOR_NAME[name] = dm._CUSTOM_DVE_ROW_BASE + len(OPS) - 1
    return op


def _split_waits(nc, maxw=1):
    """walrus here rejects >1 sync-wait per instruction; hoist extras onto
    preceding same-engine NOPs."""
    bbs = []
    for fn in nc.m.functions:
        for bb in fn.blocks:
            bbs.append((bb, list(bb.instructions)))
    new_lists = []
    for bb, insts in bbs:
        out = []
        for inst in insts:
            si = inst.sync_info
            waits = list(si.on_wait) if si and si.on_wait else []
            if len(waits) > maxw:
                chunks = [waits[i : i + maxw] for i in range(0, len(waits), maxw)]
                eng = nc.engines[inst.engine]
                for chunk in chunks[:-1]:
                    bi = eng.nop(nofuse=True)
                    ni = bi.ins if hasattr(bi, "ins") else bi
                    ni.sync_info = mybir.SyncInfo(on_wait=chunk, on_update=[])
                    out.append(ni)
                inst.sync_info = mybir.SyncInfo(
                    on_wait=chunks[-1], on_update=list(si.on_update or [])
                )
            out.append(inst)
        new_lists.append((bb, out))
    for bb, out in new_lists:
        bb.instructions = out


def _emit_quant(nc, qop, maskt, xd, nblocks, amax, out_ap):
    """NVFP4 fake-quant of SBUF AP xd [128, nblocks*16] f32 into out_ap."""
    xd = xd[:, :]
    nc.vector.tensor_reduce(
        amax[:, :],
        xd.rearrange("p (b s) -> p b s", s=16),
        axis=mybir.AxisListType.X,
        op=AO.max,
        apply_absolute_value=True,
    )
    # scale bits = ((max(amax/6, eps)) & expmask) + 1<<23   (pow2, exact)
    nc.vector.tensor_scalar(amax[:, :], amax[:, :], 1.0 / 6.0, 6e-31, AO.mult, AO.max)
    am_i = amax[:, :].bitcast(I32)
    nc.vector.tensor_scalar(am_i, am_i, MASK_EXP, None, AO.bitwise_and)
    nc.vector.tensor_scalar(am_i, am_i, 0x00800000, None, AO.add)
    nc.vector._custom_dve(
        qop,
        out=out_ap,
        in0=xd.rearrange("p (b s) -> p b s", s=16),
        in1=amax[:, :].broadcast_to([128, nblocks, 16]),
        s0=maskt[:, :],
        s1=K_MAGIC,
    )


# bisect flags (module-level so a driver can toggle before _build)
USE_PE_TRANSPOSE = True   # else: DMA-transpose + gpsimd cast (baseline style)
IN_DMA_GPSIMD = True      # else: input DMAs on the Sync queue
OUT_BF16 = True           # else: f32 output


def _build():
    qop = _get_fused_quant_op()
    nc = bass.Bass(trn_type="TRN2")
    x = nc.dram_tensor("x", [N_PER, C, H, W], F32, kind="ExternalInput")
    w = nc.dram_tensor("w", [O, C, 3, 3], F32, kind="ExternalInput")
    ident = nc.dram_tensor("ident", [128, 128], BF16, kind="ExternalInput")
    out = nc.dram_tensor(
        "out", [N_PER, O, H, W], BF16 if OUT_BF16 else F32, kind="ExternalOutput"
    )

    FPLANE = 4368  # 66*66 padded to a multiple of 16 (DoubleRow step constraint)
    ring = 3

    with TileContext(nc) as tc:
        with ExitStack() as ctx:
            wpool = ctx.enter_context(tc.tile_pool(name="wpool", bufs=1))
            lpool = ctx.enter_context(tc.tile_pool(name="lpool", bufs=1))
            xqpool = ctx.enter_context(tc.tile_pool(name="xqpool", bufs=1))
            xdpool = ctx.enter_context(tc.tile_pool(name="xdpool", bufs=4))
            smpool = ctx.enter_context(tc.tile_pool(name="smpool", bufs=2))
            obpool = ctx.enter_context(tc.tile_pool(name="obpool", bufs=8))
            pspool = ctx.enter_context(tc.tile_pool(name="ps", bufs=5, space="PSUM"))
            tppool = ctx.enter_context(tc.tile_pool(name="tp", bufs=1, space="PSUM"))

            ieng = nc.gpsimd if IN_DMA_GPSIMD else nc.sync

            maskt = wpool.tile([128, 1], F32, name="maskt", tag="maskt")
            nc.vector.memset(maskt[:, :].bitcast(I32), MASK_EXP)

            idt2 = wpool.tile([128, 128], BF16, name="idt2", tag="idt2")

            # ---- input DMAs all ride the GpSimd queue (nothing else runs
            # there), so output stores on Sync can't head-of-line block them.
            wf = [None, None]
            for oc in range(2):
                wf[oc] = wpool.tile([128, KF], F32, name=f"wf{oc}", tag=f"wf{oc}")
            xds = {}

            def emit_x_dma(n, c, halves=False):
                xd = xdpool.tile([128, F], F32, name=f"xd_{n}_{c}", tag="xd")
                src = x[n, c * 128 : (c + 1) * 128, :, :].rearrange("c h w -> c (h w)")
                if halves:
                    ieng.dma_start(out=xd[:, 0 : H0 * W], in_=src[:, 0 : H0 * W])
                    ieng.dma_start(out=xd[:, H0 * W : F], in_=src[:, H0 * W : F])
                else:
                    ieng.dma_start(out=xd[:, :], in_=src)
                xds[(n, c)] = xd

            # xq ring tiles; zero image-0's border first (tiny, no deps),
            # the other rings' borders after the early DMA issues
            xq_tiles = []

            def emit_border_memset(t):
                tv = t[:, :, 0 : HP * WP].rearrange("p c (h w) -> p c h w", h=HP)
                nc.gpsimd.memset(tv[:, :, 0, :], 0.0)
                nc.gpsimd.memset(tv[:, :, HP - 1, :], 0.0)
                nc.gpsimd.memset(tv[:, :, 1 : HP - 1, 0], 0.0)
                nc.gpsimd.memset(tv[:, :, 1 : HP - 1, WP - 1], 0.0)

            for r in range(ring):
                t = xqpool.tile([128, 2, FPLANE], FP8, name=f"xq{r}", tag=f"xq{r}")
                xq_tiles.append(t)

            # strict priority order on the single input queue: w0 in two
            # ic-halves, the x0 top row-halves, ident, then the prefetches
            ieng.dma_start(out=idt2[:, :], in_=ident[:, :])
            wsrc0 = w[0:128, :, :, :].rearrange("o i kh kw -> o (i kh kw)")
            KF4 = KF // 4
            for k in range(4):
                ieng.dma_start(
                    out=wf[0][:, k * KF4 : (k + 1) * KF4],
                    in_=wsrc0[:, k * KF4 : (k + 1) * KF4],
                )
            xd00 = xdpool.tile([128, F], F32, name="xd_0_0", tag="xd")
            xd01 = xdpool.tile([128, F], F32, name="xd_0_1", tag="xd")
            xds[(0, 0)], xds[(0, 1)] = xd00, xd01
            srcs = [
                x[0, c * 128 : (c + 1) * 128, :, :].rearrange("c h w -> c (h w)")
                for c in range(2)
            ]
            for c, xd in ((0, xd00), (1, xd01)):
                ieng.dma_start(out=xd[:, 0 : H0 * W], in_=srcs[c][:, 0 : H0 * W])
            emit_border_memset(xq_tiles[0])
            for c, xd in ((0, xd00), (1, xd01)):
                ieng.dma_start(out=xd[:, H0 * W : F], in_=srcs[c][:, H0 * W : F])
            ieng.dma_start(
                out=wf[1][:, :],
                in_=w[128:256, :, :, :].rearrange("o i kh kw -> o (i kh kw)"),
            )
            emit_x_dma(1, 0)
            emit_x_dma(1, 1)
            for r in range(1, ring):
                emit_border_memset(xq_tiles[r])
            for n in (2, 3):
                for c in range(2):
                    emit_x_dma(n, c)

            # ---- weight quant: prescale x256 on ScalarE, quant on DVE (fp8
            # out), then PE-transpose into DoubleRow lhsT tiles.
            wq = [None, None]

            def emit_w_quant(oc, quarters=False):
                if not quarters:
                    wqd = wpool.tile([128, KF], BF16, name=f"wqd{oc}", tag=f"wqd{oc}")
                    wam = smpool.tile([128, WNB], F32, name=f"wam{oc}", tag="amax")
                    _emit_quant(nc, qop, maskt, wf[oc], WNB, wam, wqd[:, :])
                    wq[oc] = (wqd, None)  # single-tile: slice by ic at use site
                    return
                # two ic-half tiles so the transposes' strided reads depend
                # only on their half (region tracking is per-tile)
                h = KF // 2
                wqa = wpool.tile([128, h], BF16, name=f"wqa{oc}", tag=f"wqa{oc}")
                wqb = wpool.tile([128, h], BF16, name=f"wqb{oc}", tag=f"wqb{oc}")
                for k, dst in ((0, wqa), (1, wqa), (2, wqb), (3, wqb)):
                    wam = smpool.tile(
                        [128, WNB // 4], F32, name=f"wam{oc}_{k}", tag="amax"
                    )
                    o = (k % 2) * (h // 2)
                    _emit_quant(
                        nc, qop, maskt, wf[oc][:, k * (h // 2) : (k + 1) * (h // 2)],
                        WNB // 4, wam, dst[:, o : o + h // 2],
                    )
                wq[oc] = (wqa, wqb)

            lhsT = {}
            tps = [
                tppool.tile([128, 1024], BF16, name="tpA", tag="tpA"),
                tppool.tile([128, 1024], BF16, name="tpB", tag="tpB"),
                tppool.tile([128, 1024], BF16, name="tpC", tag="tpC"),
            ]

            def emit_w_transposes(oc):
                wqa, wqb = wq[oc]
                if wqb is None:
                    s9 = wqa[:, :].rearrange("p (i k) -> p k i", k=9)
                    srcs9 = [s9[:, :, 0:128], s9[:, :, 128:256]]
                else:
                    srcs9 = [
                        t[:, :].rearrange("p (i k) -> p k i", k=9)
                        for t in (wqa, wqb)
                    ]
                ltall = lpool.tile(
                    [128, 9, 2, 128], FP8, name=f"lt{oc}", tag=f"lt{oc}"
                )
                for t9 in range(9):
                    lhsT[(t9 // 3, t9 % 3, oc)] = ltall[:, t9, :, :]
                for ic in range(2):
                    for t9 in range(9):
                        tp = tps[t9 // 4]
                        r = (t9 % 4) * 256
                        nc.tensor.matmul(
                            tp[:, r + ic * 128 : r + ic * 128 + 128],
                            srcs9[ic][:, t9, :],
                            idt2[:, :],
                            is_transpose=True,
                        )
                # three batched PSUM->SBUF copies (one per bank); the x256
                # weight prescale rides the activation scale
                for b, (t0, t1) in enumerate(((0, 4), (4, 8), (8, 9))):
                    nc.scalar.activation(
                        ltall[:, t0:t1, :, :], tps[b][:, 0 : (t1 - t0) * 256],
                        mybir.ActivationFunctionType.Copy, scale=256.0,
                    )

            emit_w_quant(0, quarters=True)
            emit_w_transposes(0)

            # n=0 quant in row-slices (10/24/30 rows) so the first conv
            # group only waits for rows 0..10; oc=1 weight quant rides after
            xq0 = xq_tiles[0]
            xq0v = xq0[:, :, 0 : HP * WP].rearrange("p c (h w) -> p c h w", h=HP)
            for r0, r1 in ((0, 10), (10, H0), (H0, 50), (50, H)):
                for c in range(2):
                    nb = (r1 - r0) * 4
                    am = smpool.tile(
                        [128, nb], F32, name=f"amax_0_{c}_{r0}", tag="amax"
                    )
                    _emit_quant(
                        nc, qop, maskt, xds[(0, c)][:, r0 * W : r1 * W], nb, am,
                        xq0v[:, c, r0 + 1 : r1 + 1, 1 : W + 1],
                    )
            emit_w_quant(1)

            # ---- main loop: bank-sequential implicit GEMM ----
            def emit_group(n, oc, hb, xqv):
                ps = pspool.tile([128, 512], F32, name=f"ps_{n}_{oc}_{hb}", tag="ps")
                k = 0
                for kh in range(3):
                    for kw in range(3):
                        rhs = xqv[:, :, hb * 8 + kh : hb * 8 + kh + 8, kw : kw + 64]
                        nc.tensor.matmul(
                            ps[:, :],
                            lhsT[(kh, kw, oc)][:, :, :],
                            rhs,
                            start=(k == 0),
                            stop=(k == 8),
                            perf_mode=mybir.MatmulPerfMode.DoubleRow,
                        )
                        k += 1
                ob = obpool.tile([128, 512], BF16, name=f"ob_{n}_{oc}_{hb}", tag="ob")
                # descale the 2^8 weight prescale on the way out
                nc.scalar.activation(
                    ob[:, :], ps[:, :],
                    mybir.ActivationFunctionType.Copy, scale=1.0 / 256.0,
                )
                nc.sync.dma_start(
                    out=out[n, oc * 128 : (oc + 1) * 128, hb * 8 : hb * 8 + 8, :],
                    in_=ob[:, :].rearrange("p (h w) -> p h w", h=8),
                )

            for n in range(N_PER):
                xq = xq_tiles[n % ring]
                xqv = xq[:, :, 0 : HP * WP].rearrange("p c (h w) -> p c h w", h=HP)
                if n > 0:
                    for r0, r1 in ((0, H0), (H0, H)):
                        for c in range(2):
                            nb = (r1 - r0) * 4
                            amax = smpool.tile(
                                [128, nb], F32, name=f"amax_{n}_{c}_{r0}", tag="amax"
                            )
                            _emit_quant(
                                nc, qop, maskt,
                                xds[(n, c)][:, r0 * W : r1 * W], nb, amax,
                                xqv[:, c, r0 + 1 : r1 + 1, 1 : W + 1],
                            )
                for oc in range(2):
                    for hb in range(8):
                        emit_group(n, oc, hb, xqv)
                    if n == 0 and oc == 0:
                        # oc=1 lhsT prep rides after image 0's first chunk
                        emit_w_transposes(1)

    mybir.codegen_inst_isa_subclasses(nc)
    _split_waits(nc, maxw=1)
    return nc


_NC_CACHE = None


def _get_nc():
    global _NC_CACHE
    if _NC_CACHE is None:
        _NC_CACHE = _build()
    return _NC_CACHE


def _ident_np():
    import ml_dtypes

    return np.eye(128, dtype=np.float32).astype(ml_dtypes.bfloat16)


def _in_maps(x, w):
    ident = _ident_np()
    return [
        {"x": x[i * N_PER : (i + 1) * N_PER], "w": w, "ident": ident}
        for i in range(N_CORES)
    ]


def kernel(x: np.ndarray, w: np.ndarray) -> np.ndarray:
    x = np.ascontiguousarray(x, dtype=np.float32)
    w = np.ascontiguousarray(w, dtype=np.float32)
    nc = _get_nc()
    res = run_bass_kernel_spmd(nc, _in_maps(x, w), core_ids=list(range(N_CORES)))
    outs = [
        np.asarray(res.results[i]["out"]).astype(np.float32)
        for i in range(N_CORES)
    ]
    return np.concatenate(outs, axis=0)
